# revision 8
# baseline (speedup 1.0000x reference)
"""Trainium2 Bass kernel for nn_CustomConv2d: 3x3 conv, stride 1, pad 1 (v2).

x: [32, 128, 56, 56] f32, kernel: [256, 128, 3, 3] f32, bias: [256] f32
-> out: [32, 256, 56, 56] f32

Data-parallel over batch (4 images per core on 8 cores).

Layout: x is stored in SBUF as three kw-shifted copies, each [128, 58, 56]
bf16 with 56-contiguous rows, flattened to [128, 9744] per image. A conv tap
(kh, kw) over a flat pixel tile [p0, p0+n) then reads a fully CONTIGUOUS
window at offset kw*3248 + kh*56 + p0, so the conv is 9 pure GEMM streams
per pixel tile. Pixel tiles are ragged 6x512 + 64 (full-PSUM-bank matmuls);
psum tiles are padded to a full 2KB bank.

Per (image, pixel tile, co_block): 9 accumulating bf16 matmuls into one psum
bank, then DVE adds bias and converts to bf16; gpsimd DMAs the tile out.
The host converts the bf16 output back to f32 (rel err ~2.3e-3, within the
2e-2 gate; bf16 halves both store DMA and result-fetch bytes).
"""

import sys

import numpy as np

try:
    import concourse  # noqa: F401
except ImportError:
    sys.path.insert(0, "/opt/trn_rl_repo")

import concourse.bass as bass
import concourse.mybir as mybir
import concourse.tile as tile
from concourse import bacc
from concourse.bass_utils import run_bass_kernel_spmd

B, C_IN, C_OUT, KS, H, W = 32, 128, 256, 3, 56, 56
N_CORES = 8
B_LOC = B // N_CORES
HP = H + 2  # 58 padded rows
NPIX = H * W  # 3136
CO_BLOCKS = C_OUT // 128
COPY = HP * W  # 3248 elements per kw-copy
XLEN = 3 * COPY  # 9744
# Ragged pixel tiling: full-bank 512-px matmuls stream ~2x faster than 448.
PIX_TILES = [(p0, min(512, NPIX - p0)) for p0 in range(0, NPIX, 512)]

XPOOL_BUFS = 5
OPOOL_BUFS = 6
STORE_ENG = "pool"
EXPL_LDW = False  # explicit ldweights before each matmul (PE pipelines LDW)
OUT_BF16 = True  # store output as bf16 (half DMA), convert to f32 on host

_NC_CACHE: dict = {}


def _build_cached(repeats: int = 1) -> bass.Bass:
    if repeats not in _NC_CACHE:
        _NC_CACHE[repeats] = _build(repeats)
    return _NC_CACHE[repeats]


def _build(repeats: int = 1) -> bass.Bass:
    f32 = mybir.dt.float32
    bf16 = mybir.dt.bfloat16

    nc = bacc.Bacc("TRN2", target_bir_lowering=False, debug=False)
    xp_d = nc.dram_tensor("xp", [B_LOC, C_IN, XLEN], bf16, kind="ExternalInput").ap()
    w_d = nc.dram_tensor("w", [C_IN, 9 * C_OUT], bf16, kind="ExternalInput").ap()
    b_d = nc.dram_tensor("bias", [128, CO_BLOCKS], f32, kind="ExternalInput").ap()
    out_dt = bf16 if OUT_BF16 else f32
    out_d = nc.dram_tensor("out", [B_LOC, C_OUT, H, W], out_dt, kind="ExternalOutput").ap()
    out_flat = out_d.rearrange("b c h w -> b c (h w)")

    with tile.TileContext(nc) as tc:
        with (
            tc.tile_pool(name="const", bufs=1) as const,
            tc.tile_pool(name="xpool", bufs=XPOOL_BUFS) as xpool,
            tc.tile_pool(name="opool", bufs=OPOOL_BUFS) as opool,
            tc.tile_pool(name="psum", bufs=8, space="PSUM") as psum,
        ):
            import contextlib

            loop_cm = (
                tc.For_i(0, repeats, 1, hint_engines=(mybir.EngineType.PE,),
                         staggered_reset=True)
                if repeats > 1
                else contextlib.nullcontext()
            )
            with loop_cm:
                wco = [
                    const.tile([C_IN, 9 * 128], bf16, tag=f"w{co}", name=f"w{co}")
                    for co in range(CO_BLOCKS)
                ]
                bt = const.tile([128, CO_BLOCKS], f32)
                nc.sync.dma_start(wco[0][:], w_d[:, : 9 * 128])
                xc_up = []
                for b in range(B_LOC):
                    xt = xpool.tile([C_IN, XLEN], bf16, tag="xw", name="xw")
                    nc.sync.dma_start(xt[:], xp_d[b])
                    xc_up.append(xt)
                    if b == 0:
                        nc.sync.dma_start(wco[1][:], w_d[:, 9 * 128 :])
                nc.sync.dma_start(bt[:], b_d[:])

                for b in range(B_LOC):
                    xt = xc_up[b]
                    for p0, n in PIX_TILES:
                        for co in range(CO_BLOCKS):
                            pt = psum.tile([128, 512], f32, tag="pt", name="pt")
                            for k in range(9):
                                kh, kw = divmod(k, KS)
                                off = kw * COPY + kh * W + p0
                                w_ap = wco[co][:, k * 128 : (k + 1) * 128]
                                if EXPL_LDW:
                                    nc.tensor.ldweights(w_ap)
                                nc.tensor.matmul(
                                    pt[:, :n],
                                    w_ap,
                                    xt[:, off : off + n],
                                    start=(k == 0),
                                    stop=(k == 8),
                                )
                            ot = opool.tile([128, 512], out_dt)
                            nc.vector.tensor_scalar_add(
                                ot[:, :n], pt[:, :n], bt[:, co : co + 1]
                            )
                            store_eng = nc.gpsimd if STORE_ENG == "pool" else nc.scalar
                            store_eng.dma_start(
                                out_flat[b, co * 128 : (co + 1) * 128, p0 : p0 + n],
                                ot[:, :n],
                            )
    nc.compile()
    return nc


def _host_prep(x, kernel, bias):
    import ml_dtypes

    bf = ml_dtypes.bfloat16
    xp = np.zeros((B, C_IN, HP, HP), dtype=np.float32)
    xp[:, :, 1 : 1 + H, 1 : 1 + W] = x
    xs = np.empty((B, C_IN, 3, HP, W), dtype=bf)
    for kw in range(3):
        xs[:, :, kw] = xp[:, :, :, kw : kw + W]
    xs = xs.reshape(B, C_IN, XLEN)
    # w[co, ci, kh, kw] -> w_t[ci, co_blk*9*128 + (kh*3+kw)*128 + co_in]
    w5 = kernel.reshape(CO_BLOCKS, 128, C_IN, KS, KS)
    w_t = np.ascontiguousarray(
        w5.transpose(2, 0, 3, 4, 1).reshape(C_IN, 9 * C_OUT).astype(bf)
    )
    b_t = np.ascontiguousarray(bias.astype(np.float32).reshape(CO_BLOCKS, 128).T)
    return xs, w_t, b_t


def kernel(x, kernel, bias):  # noqa: A002 - names fixed by harness contract
    x = np.asarray(x, dtype=np.float32)
    kernel = np.asarray(kernel, dtype=np.float32)
    bias = np.asarray(bias, dtype=np.float32)

    nc = _build_cached()
    xs, w_t, b_t = _host_prep(x, kernel, bias)
    in_maps = [
        {"xp": xs[c * B_LOC : (c + 1) * B_LOC], "w": w_t, "bias": b_t}
        for c in range(N_CORES)
    ]
    res = run_bass_kernel_spmd(nc, in_maps, core_ids=list(range(N_CORES)))
    out = np.concatenate([r["out"] for r in res.results], axis=0)
    return np.ascontiguousarray(out.astype(np.float32))
